# revision 27
# baseline (speedup 1.0000x reference)
"""Trainium2 Bass kernel for MatrixMPowerSeriesLayer.

Computes res = sum_{i=0}^{7} C_i @ X^i for a batch of 64 complex 512x512
matrices (real/imag stacked), data-parallel over batch across 8 NeuronCores.

Algorithm (per batch element):
  Transposed Horner:  G_7 = C_7^T;  G_k = C_k^T + X^T @ G_{k+1}  (k = 6..0)
  => G_0 = res^T.  On the PE, out = lhsT.T @ rhs, so X^T @ G needs lhsT = X
  (untransposed!) and rhs = G: no transposes on device at all.  Host feeds
  coefficients transposed and transposes the result back.

  Mixed precision across the Horner chain: an error injected at step k is
  multiplied by X^T on every later step (spectral norm ~0.9, Frobenius gain
  ~0.45 on incoherent noise), so the EARLY steps tolerate coarse arithmetic.
  Numerically validated split (simulated rel_l2 7.8e-3 vs the 2e-2 budget):

  * Steps k = 6..3 run in fp8-e4m3 with perf_mode=DoubleRow (2 fp8 weights
    per PE cell -> K=256 per matmul, ~1.8x PE throughput) using the plain
    4M complex product (no Gs state needed):
      t_rr = Xr^T Gr   t_ii = Xi^T Gi   t_ri = Xr^T Gi   t_ir = Xi^T Gr
      Gr'  = Cr^T + t_rr - t_ii         Gi' = Ci^T + t_ri + t_ir
    X is pre-scaled by 32 into e4m3 (its ~0.02 entries sit in the subnormal
    range otherwise); the 1/32 rides the ScalarE PSUM-evacuation copies and
    the scalar slot of the VectorE scalar_tensor_tensor ops.  The combine
    writes the next fp8 G-state directly; step 3 instead emits the fp16
    state (and Gs) consumed by the fp16 tail.
  * Steps k = 2..0 run in fp16 with Karatsuba (3 real matmuls):
      T1 = Xr^T Gr, T2 = Xi^T Gi, T3 = (Xr+Xi)^T (Gr+Gi)
      Gr' = Cr^T + T1 - T2,  Gi' = Ci^T + T3 - T1 - T2,  Gs' = Gr' + Gi'
    ScalarE stages each PSUM bank to SBUF fp16; VectorE runs the combine as
    all-fp16 ops (DVE 2x_1P mode).

  Two batch elements interleave at the Horner-step level so the PE never
  waits on a combine tail at a step boundary; fp16 operands give the PE
  FastWeightLoad (fp32 disables it) so LDWEIGHTS hides under the previous
  matmul's stream.
"""

import numpy as np
from contextlib import ExitStack

import concourse.bass as bass
from concourse import bacc
import concourse.mybir as mybir
import concourse.tile as tile
from concourse.bass_utils import run_bass_kernel_spmd

B, N, DEG = 64, 512, 8
P = 128
KO = N // P          # 4 partition-chunks per 512 dim
NCORES = 8
BPC = B // NCORES    # 8 batch elements per core
F32 = mybir.dt.float32
F16 = mybir.dt.float16
F8 = mybir.dt.float8e4
XSCALE = 32.0        # fp8 pre-scale on X (entries ~0.02 are subnormal in e4m3)
# Error budget: each step's fp8 noise is attenuated by later X^T multiplies;
# simulated rel_l2 = 1.25e-2 for 5 fp8 steps (vs 7.8e-3 for 4, gate 2e-2),
# and HW matched the simulation to <0.5% on both configs (measured 1.2515e-2
# / 7.807e-3).  Fallback knob: drop 2 (or also 3) from FP8_STEPS for more
# accuracy at ~21us (42us) cost; the rest of the code adapts automatically.
FP8_STEPS = {6, 5, 4, 3, 2}

_NC_CACHE: dict = {}


def _build_nc(bpc: int = BPC, deg: int = DEG) -> bass.Bass:
    """Build the per-core Bass program (SPMD; same program on all cores)."""
    nc = bacc.Bacc()
    DR = mybir.MatmulPerfMode.DoubleRow

    # DRAM inputs (per core).  Layout [P, KO, N]: matrix row r lives at
    # [r % 128, r // 128, :], so each partition line is contiguous and a
    # whole matrix moves in ONE dma.
    xr8_d = nc.declare_dram_parameter("xr8", [bpc, P, KO, N], F8, isOutput=False)
    xi8_d = nc.declare_dram_parameter("xi8", [bpc, P, KO, N], F8, isOutput=False)
    xr_d = nc.declare_dram_parameter("xr", [bpc, P, KO, N], F16, isOutput=False)
    xi_d = nc.declare_dram_parameter("xi", [bpc, P, KO, N], F16, isOutput=False)
    xs_d = nc.declare_dram_parameter("xs", [bpc, P, KO, N], F16, isOutput=False)
    # Coefficients, j = deg-2-k (j=0 is the first Horner step k=deg-2)
    ctr_d = nc.declare_dram_parameter("ctr", [deg - 1, P, KO, N], F16, isOutput=False)
    cti_d = nc.declare_dram_parameter("cti", [deg - 1, P, KO, N], F16, isOutput=False)
    # Initial state G_{deg-1} = C_{deg-1}^T as fp8: [r, i]
    g08_d = nc.declare_dram_parameter("g08", [2, P, KO, N], F8, isOutput=False)
    id_d = nc.declare_dram_parameter("ident", [P, P], F16, isOutput=False)

    or_d = nc.declare_dram_parameter("o_r", [bpc, KO, P, N], F16, isOutput=True)
    oi_d = nc.declare_dram_parameter("o_i", [bpc, KO, P, N], F16, isOutput=True)

    with tile.TileContext(nc) as tc, ExitStack() as ctx:
        xp = ctx.enter_context(tc.tile_pool(name="xp", bufs=2))
        gp = ctx.enter_context(tc.tile_pool(name="gp", bufs=2))
        kp = ctx.enter_context(tc.tile_pool(name="kp", bufs=1))
        sp = ctx.enter_context(tc.tile_pool(name="sp", bufs=4))
        vp = ctx.enter_context(tc.tile_pool(name="vp", bufs=2))
        ps = ctx.enter_context(tc.tile_pool(name="ps", bufs=2, space="PSUM"))

        ct_r = kp.tile([P, deg - 1, KO, N], F16, name="ct_r")
        ct_i = kp.tile([P, deg - 1, KO, N], F16, name="ct_i")
        g08_t = kp.tile([P, 2, KO, N], F8, name="g08_t")
        ident = kp.tile([P, P], F16, name="ident")
        nc.sync.dma_start(out=ident[:], in_=id_d[:])

        # PE warmup: garbage matmuls bridge the DMA prologue so the HAM clock
        # gate starts opening (needs ~3.4us of PE busy) before real work.
        wu = kp.tile([P, N], F16, name="wu")
        nc.vector.memset(wu[:], 0.0)
        wps = ps.tile([P, N], F32, tag="t1", name="wps")
        for _ in range(8):
            nc.tensor.matmul(wps[:], lhsT=wu[:, 0:P], rhs=wu[:], start=True, stop=True)

        for pair in range(bpc // 2):
            b0, b1 = 2 * pair, 2 * pair + 1
            xts = {}
            for b in (b0, b1):
                par = b % 2
                xr8_t = xp.tile([P, KO, N], F8, tag=f"xr8{par}", name=f"xr8{b}")
                xi8_t = xp.tile([P, KO, N], F8, tag=f"xi8{par}", name=f"xi8{b}")
                nc.sync.dma_start(out=xr8_t[:], in_=xr8_d[b])
                if pair == 0 and b == b0:
                    nc.sync.dma_start(out=g08_t[:, 0], in_=g08_d[0])
                    nc.sync.dma_start(out=ct_r[:, 0], in_=ctr_d[0])
                nc.sync.dma_start(out=xi8_t[:], in_=xi8_d[b])
                if pair == 0 and b == b0:
                    nc.sync.dma_start(out=g08_t[:, 1], in_=g08_d[1])
                    nc.sync.dma_start(out=ct_i[:, 0], in_=cti_d[0])
                xts[b] = [xr8_t, xi8_t, None, None, None]
            # fp16 X tiles (needed from step k=2; transfers hide under the
            # fp8 phase).  Xs comes from the host too: computing it on-device
            # (VectorE or GpSimdE) stretches concurrent DVE ops via SBUF-port
            # contention and stalls the PE at pair starts.
            for b in (b0, b1):
                par = b % 2
                xr_t = xp.tile([P, KO, N], F16, tag=f"xr{par}", name=f"xr{b}")
                xi_t = xp.tile([P, KO, N], F16, tag=f"xi{par}", name=f"xi{b}")
                xs_t = xp.tile([P, KO, N], F16, tag=f"xs{par}", name=f"xs{b}")
                nc.sync.dma_start(out=xr_t[:], in_=xr_d[b])
                nc.sync.dma_start(out=xi_t[:], in_=xi_d[b])
                nc.sync.dma_start(out=xs_t[:], in_=xs_d[b])
                xts[b][2:5] = [xr_t, xi_t, xs_t]
            if pair == 0:
                # Remaining coefficient steps stream in behind the X tiles.
                for j in range(1, deg - 1):
                    nc.sync.dma_start(out=ct_r[:, j], in_=ctr_d[j])
                    nc.sync.dma_start(out=ct_i[:, j], in_=cti_d[j])

            # Current G state per element: (gr8, gi8) during the fp8 phase,
            # (gr, gi, gs) fp16 afterwards.  k = deg-1 reads from g08_t.
            gcur = {b0: None, b1: None}

            for k in range(deg - 2, -1, -1):
                j = deg - 2 - k
                for b in (b0, b1):
                    par = b % 2
                    xr8_t, xi8_t, xr_t, xi_t, xs_t = xts[b]
                    if k in FP8_STEPS:
                        to_fp16 = (k - 1) not in FP8_STEPS
                        if gcur[b] is None:
                            r8 = lambda h: g08_t[:, 0, 2 * h : 2 * h + 2, :]
                            i8 = lambda h: g08_t[:, 1, 2 * h : 2 * h + 2, :]
                        else:
                            _gr8, _gi8 = gcur[b]
                            r8 = lambda h, t=_gr8: t[:, 2 * h : 2 * h + 2, :]
                            i8 = lambda h, t=_gi8: t[:, 2 * h : 2 * h + 2, :]
                        if to_fp16:
                            gr_n = gp.tile([P, KO, N], F16, tag=f"gr{par}", name=f"gr{b}_{k}")
                            gi_n = gp.tile([P, KO, N], F16, tag=f"gi{par}", name=f"gi{b}_{k}")
                            gs_n = gp.tile([P, KO, N], F16, tag=f"gs{par}", name=f"gs{b}_{k}")
                        else:
                            gr_n = gp.tile([P, KO, N], F8, tag=f"gr8{par}", name=f"gr8{b}_{k}")
                            gi_n = gp.tile([P, KO, N], F8, tag=f"gi8{par}", name=f"gi8{b}_{k}")

                        for m in range(KO):
                            msl = slice(m * P, (m + 1) * P)
                            t_rr = ps.tile([P, N], F32, tag="t1", name=f"trr_{b}_{k}_{m}")
                            t_ii = ps.tile([P, N], F32, tag="t2", name=f"tii_{b}_{k}_{m}")
                            t_ri = ps.tile([P, N], F32, tag="t3", name=f"tri_{b}_{k}_{m}")
                            t_ir = ps.tile([P, N], F32, tag="t4", name=f"tir_{b}_{k}_{m}")
                            for dst, x_t, rhs_fn in (
                                (t_rr, xr8_t, r8),
                                (t_ii, xi8_t, i8),
                                (t_ri, xr8_t, i8),
                                (t_ir, xi8_t, r8),
                            ):
                                for h in range(2):
                                    nc.tensor.matmul(
                                        dst[:],
                                        lhsT=x_t[:, 2 * h : 2 * h + 2, msl],
                                        rhs=rhs_fn(h),
                                        start=(h == 0),
                                        stop=(h == 1) and dst is not t_rr,
                                        perf_mode=DR,
                                    )
                                if dst is t_rr:
                                    # Seed the rr bank with 32*Cr^T (identity
                                    # matmul; ctr_d rows for fp8 steps are
                                    # host-prescaled by XSCALE): keeps the
                                    # tensor-add off the bottleneck VectorE,
                                    # and running it AFTER the DR pair lets
                                    # the first chunk start before the (later)
                                    # coefficient DMA lands.
                                    nc.tensor.matmul(
                                        t_rr[:], lhsT=ident[:], rhs=ct_r[:, j, m, :],
                                        start=False, stop=True,
                                    )

                            # ScalarE evacuates the rr/ri banks (folding the
                            # 1/XSCALE); GpSimdE adds Ci; VectorE reads the
                            # ii/ir banks straight from PSUM via
                            # scalar_tensor_tensor.
                            s_rr = sp.tile([P, N], F16, tag="t1s", name=f"srr_{b}_{k}_{m}")
                            s_ri = sp.tile([P, N], F16, tag="t2s", name=f"sri_{b}_{k}_{m}")
                            nc.scalar.activation(
                                s_rr[:], t_rr[:],
                                mybir.ActivationFunctionType.Copy, scale=1.0 / XSCALE,
                            )
                            nc.scalar.activation(
                                s_ri[:], t_ri[:],
                                mybir.ActivationFunctionType.Copy, scale=1.0 / XSCALE,
                            )
                            # On the transition step the chunk also computes
                            # Gs on GpSimdE, so Ci's add moves to VectorE to
                            # keep the GpSimd/DVE SBUF-port overlap minimal.
                            v = vp.tile([P, N], F16, tag="v2", name=f"v_{b}_{k}_{m}")
                            veng = nc.vector if to_fp16 else nc.gpsimd
                            veng.tensor_add(v[:], s_ri[:], ct_i[:, j, m, :])
                            nc.vector.scalar_tensor_tensor(
                                gr_n[:, m, :], t_ii[:], -1.0 / XSCALE, s_rr[:],
                                op0=mybir.AluOpType.mult, op1=mybir.AluOpType.add,
                            )
                            nc.vector.scalar_tensor_tensor(
                                gi_n[:, m, :], t_ir[:], 1.0 / XSCALE, v[:],
                                op0=mybir.AluOpType.mult, op1=mybir.AluOpType.add,
                            )
                            if to_fp16:
                                nc.gpsimd.tensor_add(
                                    gs_n[:, m, :], gr_n[:, m, :], gi_n[:, m, :]
                                )
                        gcur[b] = (gr_n, gi_n, gs_n) if to_fp16 else (gr_n, gi_n)
                        continue

                    # ---- fp16 Karatsuba step (k = 2..0) ----
                    last = k == 0
                    _gr, _gi, _gs = gcur[b]
                    gr_n = gp.tile([P, KO, N], F16, tag=f"gr{par}", name=f"gr{b}_{k}")
                    gi_n = gp.tile([P, KO, N], F16, tag=f"gi{par}", name=f"gi{b}_{k}")
                    gs_n = (
                        None
                        if last
                        else gp.tile([P, KO, N], F16, tag=f"gs{par}", name=f"gs{b}_{k}")
                    )

                    for m in range(KO):
                        msl = slice(m * P, (m + 1) * P)
                        t1 = ps.tile([P, N], F32, tag="t1", name=f"t1_{b}_{k}_{m}")
                        t2 = ps.tile([P, N], F32, tag="t2", name=f"t2_{b}_{k}_{m}")
                        t3 = ps.tile([P, N], F32, tag="t3", name=f"t3_{b}_{k}_{m}")
                        for dst, x_t, g_t in ((t1, xr_t, _gr), (t2, xi_t, _gi), (t3, xs_t, _gs)):
                            for ko in range(KO):
                                nc.tensor.matmul(
                                    dst[:],
                                    lhsT=x_t[:, ko, msl],
                                    rhs=g_t[:, ko, :],
                                    start=(ko == 0),
                                    stop=(ko == KO - 1),
                                )

                        t1s = sp.tile([P, N], F16, tag="t1s", name=f"t1s_{b}_{k}_{m}")
                        t2s = sp.tile([P, N], F16, tag="t2s", name=f"t2s_{b}_{k}_{m}")
                        t3s = sp.tile([P, N], F16, tag="t3s", name=f"t3s_{b}_{k}_{m}")
                        nc.scalar.copy(t1s[:], t1[:])
                        nc.scalar.copy(t2s[:], t2[:])
                        nc.scalar.copy(t3s[:], t3[:])

                        v1 = vp.tile([P, N], F16, tag="v1", name=f"v1_{b}_{k}_{m}")
                        v2 = vp.tile([P, N], F16, tag="v2", name=f"v2_{b}_{k}_{m}")
                        w2 = vp.tile([P, N], F16, tag="w2", name=f"w2_{b}_{k}_{m}")
                        nc.vector.tensor_sub(v1[:], t1s[:], t2s[:])
                        nc.vector.tensor_add(gr_n[:, m, :], v1[:], ct_r[:, j, m, :])
                        nc.vector.tensor_sub(v2[:], t3s[:], t1s[:])
                        nc.vector.tensor_sub(w2[:], v2[:], t2s[:])
                        nc.vector.tensor_add(gi_n[:, m, :], w2[:], ct_i[:, j, m, :])
                        if last:
                            nc.sync.dma_start(out=or_d[b, m], in_=gr_n[:, m, :])
                            nc.sync.dma_start(out=oi_d[b, m], in_=gi_n[:, m, :])
                        else:
                            nc.vector.tensor_add(
                                gs_n[:, m, :], gr_n[:, m, :], gi_n[:, m, :]
                            )

                    gcur[b] = (gr_n, gi_n, gs_n)

    nc.finalize()
    return nc


def _get_nc() -> bass.Bass:
    if "nc" not in _NC_CACHE:
        _NC_CACHE["nc"] = _build_nc()
    return _NC_CACHE["nc"]


def _tile_layout(m: np.ndarray) -> np.ndarray:
    """[N, N] row-major -> [P, KO, N] (row r at [r % P, r // P, :])."""
    return np.ascontiguousarray(m.reshape(KO, P, N).transpose(1, 0, 2))


def _prep_inputs(x: np.ndarray, coeffs: np.ndarray):
    """Host-side prep: tile/transpose into the DRAM layouts the kernel wants."""
    import ml_dtypes

    f8 = ml_dtypes.float8_e4m3
    x = np.ascontiguousarray(x, dtype=np.float32)
    coeffs = np.ascontiguousarray(coeffs, dtype=np.float32)

    # [B, P, KO, N]
    xrf = x[:, 0].reshape(B, KO, P, N).transpose(0, 2, 1, 3)
    xif = x[:, 1].reshape(B, KO, P, N).transpose(0, 2, 1, 3)
    xr = np.ascontiguousarray(xrf.astype(np.float16))
    xi = np.ascontiguousarray(xif.astype(np.float16))
    xs = np.ascontiguousarray((xrf + xif).astype(np.float16))
    xr8 = np.ascontiguousarray((xrf * XSCALE).astype(f8))
    xi8 = np.ascontiguousarray((xif * XSCALE).astype(f8))

    crT = coeffs[:, 0].transpose(0, 2, 1)  # [DEG, N, N]
    ciT = coeffs[:, 1].transpose(0, 2, 1)
    ctr = np.empty((DEG - 1, P, KO, N), dtype=np.float16)
    cti = np.empty((DEG - 1, P, KO, N), dtype=np.float16)
    for jj in range(DEG - 1):
        k = DEG - 2 - jj
        # fp8 steps (k in FP8_STEPS) read Cr^T through the XSCALE-scaled PSUM
        # seed; prescale those rows on the host.  Ci^T stays unscaled (added
        # after the 1/XSCALE evacuation).
        rs = XSCALE if k in FP8_STEPS else 1.0
        ctr[jj] = (_tile_layout(crT[k]) * rs).astype(np.float16)
        cti[jj] = _tile_layout(ciT[k]).astype(np.float16)
    g08 = np.empty((2, P, KO, N), dtype=f8)
    g08[0] = _tile_layout(crT[DEG - 1]).astype(f8)
    g08[1] = _tile_layout(ciT[DEG - 1]).astype(f8)

    in_maps = []
    for c in range(NCORES):
        sl = slice(c * BPC, (c + 1) * BPC)
        in_maps.append(
            {
                "xr8": np.ascontiguousarray(xr8[sl]),
                "xi8": np.ascontiguousarray(xi8[sl]),
                "xr": np.ascontiguousarray(xr[sl]),
                "xi": np.ascontiguousarray(xi[sl]),
                "xs": np.ascontiguousarray(xs[sl]),
                "ctr": ctr,
                "cti": cti,
                "g08": g08,
                "ident": np.eye(P, dtype=np.float16),
            }
        )
    return in_maps


def _assemble_output(results) -> np.ndarray:
    out = np.empty((B, 2, N, N), dtype=np.float32)
    for c in range(NCORES):
        o_r = results[c]["o_r"].reshape(BPC, N, N).astype(np.float32)
        o_i = results[c]["o_i"].reshape(BPC, N, N).astype(np.float32)
        for b in range(BPC):
            out[c * BPC + b, 0] = o_r[b].T
            out[c * BPC + b, 1] = o_i[b].T
    return out


def run_sharded(x: np.ndarray, coeffs: np.ndarray, **run_kwargs):
    """Run the SPMD kernel on 8 cores; returns (output, BassKernelResults)."""
    nc = _get_nc()
    in_maps = _prep_inputs(x, coeffs)
    res = run_bass_kernel_spmd(nc, in_maps, list(range(NCORES)), **run_kwargs)
    return _assemble_output(res.results), res


def kernel(x: np.ndarray, coeffs: np.ndarray) -> np.ndarray:
    out, _ = run_sharded(x, coeffs)
    return out


# revision 38
# speedup vs baseline: 1.0880x; 1.0880x over previous
"""Trainium2 Bass kernel for MatrixMPowerSeriesLayer.

Computes res = sum_{i=0}^{7} C_i @ X^i for a batch of 64 complex 512x512
matrices (real/imag stacked), data-parallel over batch across 8 NeuronCores.

Algorithm (per batch element):
  Transposed Horner:  G_7 = C_7^T;  G_k = C_k^T + X^T @ G_{k+1}  (k = 6..0)
  => G_0 = res^T.  On the PE, out = lhsT.T @ rhs, so X^T @ G needs lhsT = X
  (untransposed!) and rhs = G: no transposes on device at all.  Host feeds
  coefficients transposed and transposes the result back.

  Mixed precision across the Horner chain: an error injected at step k is
  multiplied by X^T on every later step (spectral norm ~0.9, Frobenius gain
  ~0.45 on incoherent noise), so the EARLY steps tolerate coarse arithmetic.
  Numerically validated split (simulated rel_l2 7.8e-3 vs the 2e-2 budget):

  * Steps k = 6..3 run in fp8-e4m3 with perf_mode=DoubleRow (2 fp8 weights
    per PE cell -> K=256 per matmul, ~1.8x PE throughput) using the plain
    4M complex product (no Gs state needed):
      t_rr = Xr^T Gr   t_ii = Xi^T Gi   t_ri = Xr^T Gi   t_ir = Xi^T Gr
      Gr'  = Cr^T + t_rr - t_ii         Gi' = Ci^T + t_ri + t_ir
    X is pre-scaled by 32 into e4m3 (its ~0.02 entries sit in the subnormal
    range otherwise); the 1/32 rides the ScalarE PSUM-evacuation copies and
    the scalar slot of the VectorE scalar_tensor_tensor ops.  The combine
    writes the next fp8 G-state directly; step 3 instead emits the fp16
    state (and Gs) consumed by the fp16 tail.
  * Steps k = 2..0 run in fp16 with Karatsuba (3 real matmuls):
      T1 = Xr^T Gr, T2 = Xi^T Gi, T3 = (Xr+Xi)^T (Gr+Gi)
      Gr' = Cr^T + T1 - T2,  Gi' = Ci^T + T3 - T1 - T2,  Gs' = Gr' + Gi'
    ScalarE stages each PSUM bank to SBUF fp16; VectorE runs the combine as
    all-fp16 ops (DVE 2x_1P mode).

  Two batch elements interleave at the Horner-step level so the PE never
  waits on a combine tail at a step boundary; fp16 operands give the PE
  FastWeightLoad (fp32 disables it) so LDWEIGHTS hides under the previous
  matmul's stream.
"""

import numpy as np
from contextlib import ExitStack

import concourse.bass as bass
from concourse import bacc
import concourse.mybir as mybir
import concourse.tile as tile
from concourse.bass_utils import run_bass_kernel_spmd

B, N, DEG = 64, 512, 8
P = 128
KO = N // P          # 4 partition-chunks per 512 dim
NCORES = 8
BPC = B // NCORES    # 8 batch elements per core
F32 = mybir.dt.float32
F16 = mybir.dt.float16
F8 = mybir.dt.float8e4
XSCALE = 32.0        # fp8 pre-scale on X (entries ~0.02 are subnormal in e4m3)
# Error budget: each step's fp8 noise is attenuated by later X^T multiplies;
# simulated rel_l2 = 1.25e-2 for 5 fp8 steps (vs 7.8e-3 for 4, gate 2e-2),
# and HW matched the simulation to <0.5% on both configs (measured 1.2515e-2
# / 7.807e-3).  Fallback knob: drop 2 (or also 3) from FP8_STEPS for more
# accuracy at ~21us (42us) cost; the rest of the code adapts automatically.
FP8_STEPS = {6, 5, 4, 3, 2}

_NC_CACHE: dict = {}


def _build_nc(bpc: int = BPC, deg: int = DEG) -> bass.Bass:
    """Build the per-core Bass program (SPMD; same program on all cores)."""
    nc = bacc.Bacc()
    DR = mybir.MatmulPerfMode.DoubleRow

    # DRAM inputs (per core).  Layout [P, KO, N]: matrix row r lives at
    # [r % 128, r // 128, :], so each partition line is contiguous and a
    # whole matrix moves in ONE dma.
    xr8_d = nc.declare_dram_parameter("xr8", [bpc, P, KO, N], F8, isOutput=False)
    xi8_d = nc.declare_dram_parameter("xi8", [bpc, P, KO, N], F8, isOutput=False)
    nxr8_d = nc.declare_dram_parameter("nxr8", [bpc, P, KO, N], F8, isOutput=False)
    xr_d = nc.declare_dram_parameter("xr", [bpc, P, KO, N], F16, isOutput=False)
    xi_d = nc.declare_dram_parameter("xi", [bpc, P, KO, N], F16, isOutput=False)
    xs_d = nc.declare_dram_parameter("xs", [bpc, P, KO, N], F16, isOutput=False)
    # Coefficients, j = deg-2-k (j=0 is the first Horner step k=deg-2)
    ctr_d = nc.declare_dram_parameter("ctr", [deg - 1, P, KO, N], F16, isOutput=False)
    cti_d = nc.declare_dram_parameter("cti", [deg - 1, P, KO, N], F16, isOutput=False)
    # Initial state (Gr, -Gi) = (Cr^T, -Ci^T) for deg-1, as fp8
    g08_d = nc.declare_dram_parameter("g08", [2, P, KO, N], F8, isOutput=False)

    or_d = nc.declare_dram_parameter("o_r", [bpc, KO, P, N], F16, isOutput=True)
    oi_d = nc.declare_dram_parameter("o_i", [bpc, KO, P, N], F16, isOutput=True)

    with tile.TileContext(nc) as tc, ExitStack() as ctx:
        xp = ctx.enter_context(tc.tile_pool(name="xp", bufs=2))
        gp = ctx.enter_context(tc.tile_pool(name="gp", bufs=2))
        kp = ctx.enter_context(tc.tile_pool(name="kp", bufs=1))
        sp = ctx.enter_context(tc.tile_pool(name="sp", bufs=3))
        vp = ctx.enter_context(tc.tile_pool(name="vp", bufs=2))
        ps = ctx.enter_context(tc.tile_pool(name="ps", bufs=2, space="PSUM"))

        ct_r = kp.tile([P, deg - 1, KO, N], F16, name="ct_r")
        ct_i = kp.tile([P, deg - 1, KO, N], F16, name="ct_i")
        g08_t = kp.tile([P, 2, KO, N], F8, name="g08_t")

        # PE warmup: garbage matmuls bridge the DMA prologue so the HAM clock
        # gate starts opening (needs ~3.4us of PE busy) before real work.
        wu = kp.tile([P, N], F16, name="wu")
        nc.vector.memset(wu[:], 0.0)
        wps = ps.tile([P, N], F32, tag="t1", name="wps")
        for _ in range(8):
            nc.tensor.matmul(wps[:], lhsT=wu[:, 0:P], rhs=wu[:], start=True, stop=True)

        for pair in range(bpc // 2):
            b0, b1 = 2 * pair, 2 * pair + 1
            xts = {}
            for b in (b0, b1):
                par = b % 2
                xr8_t = xp.tile([P, KO, N], F8, tag=f"xr8{par}", name=f"xr8{b}")
                xi8_t = xp.tile([P, KO, N], F8, tag=f"xi8{par}", name=f"xi8{b}")
                nxr8_t = xp.tile([P, KO, N], F8, tag=f"nxr8{par}", name=f"nxr8{b}")
                nc.sync.dma_start(out=xr8_t[:], in_=xr8_d[b])
                if pair == 0 and b == b0:
                    nc.sync.dma_start(out=g08_t[:, 0], in_=g08_d[0])
                nc.sync.dma_start(out=xi8_t[:], in_=xi8_d[b])
                if pair == 0 and b == b0:
                    nc.sync.dma_start(out=g08_t[:, 1], in_=g08_d[1])
                nc.sync.dma_start(out=nxr8_t[:], in_=nxr8_d[b])
                if pair == 0 and b == b0:
                    nc.sync.dma_start(out=ct_r[:, 0], in_=ctr_d[0])
                    nc.sync.dma_start(out=ct_i[:, 0], in_=cti_d[0])
                xts[b] = [xr8_t, xi8_t, nxr8_t, None, None, None]
            if pair == 0:
                # Remaining coefficient steps stream in before the fp16 X
                # tiles: step j's combine needs them from ~8*j us in, while
                # the fp16 tiles wait until k=2 (~70 us).
                for j in range(1, deg - 1):
                    nc.sync.dma_start(out=ct_r[:, j], in_=ctr_d[j])
                    nc.sync.dma_start(out=ct_i[:, j], in_=cti_d[j])
            # fp16 X tiles (needed from step k=2; transfers hide under the
            # fp8 phase).  Xs comes from the host too: computing it on-device
            # (VectorE or GpSimdE) stretches concurrent DVE ops via SBUF-port
            # contention and stalls the PE at pair starts.
            for b in (b0, b1):
                par = b % 2
                # Single-buffered: consumed only in the pair's back half
                # (k<=2), and the next pair's DMA refill easily beats that.
                xr_t = xp.tile([P, KO, N], F16, tag=f"xr{par}", bufs=1, name=f"xr{b}")
                xi_t = xp.tile([P, KO, N], F16, tag=f"xi{par}", bufs=1, name=f"xi{b}")
                xs_t = xp.tile([P, KO, N], F16, tag=f"xs{par}", bufs=1, name=f"xs{b}")
                nc.sync.dma_start(out=xr_t[:], in_=xr_d[b])
                nc.sync.dma_start(out=xi_t[:], in_=xi_d[b])
                nc.sync.dma_start(out=xs_t[:], in_=xs_d[b])
                xts[b][3:6] = [xr_t, xi_t, xs_t]
            # Current G state per element: (gr8, gi8) during the fp8 phase,
            # (gr, gi, gs) fp16 afterwards.  k = deg-1 reads from g08_t.
            gcur = {b0: None, b1: None}

            for k in range(deg - 2, -1, -1):
                j = deg - 2 - k
                for b in (b0, b1):
                    par = b % 2
                    xr8_t, xi8_t, nxr8_t, xr_t, xi_t, xs_t = xts[b]
                    if k in FP8_STEPS:
                        # fp8 state is (Gr, -Gi); with the host-negated -Xr
                        # tile, BOTH complex components become plain SUMS of
                        # available products and fit in TWO psum banks:
                        #   bank_re = Xr*Gr + Xi*(-Gi)   = 32(XrGr - XiGi)
                        #   bank_im = Xi*Gr + (-Xr)*(-Gi) = 32(XiGr + XrGi)
                        # so there are 8 DoubleRow matmuls, no seed, and the
                        # whole combine is 2 ScalarE stages + 2 VectorE adds.
                        to_fp16 = (k - 1) not in FP8_STEPS
                        if gcur[b] is None:
                            r8 = lambda h: g08_t[:, 0, 2 * h : 2 * h + 2, :]
                            ni8 = lambda h: g08_t[:, 1, 2 * h : 2 * h + 2, :]
                        else:
                            _gr8, _ngi8 = gcur[b]
                            r8 = lambda h, t=_gr8: t[:, 2 * h : 2 * h + 2, :]
                            ni8 = lambda h, t=_ngi8: t[:, 2 * h : 2 * h + 2, :]
                        if to_fp16:
                            gr_n = gp.tile([P, KO, N], F16, tag=f"gr{par}", name=f"gr{b}_{k}")
                            gi_n = gp.tile([P, KO, N], F16, tag=f"gi{par}", name=f"gi{b}_{k}")
                            gs_n = gp.tile([P, KO, N], F16, tag=f"gs{par}", name=f"gs{b}_{k}")
                        else:
                            gr_n = gp.tile([P, KO, N], F8, tag=f"gr8{par}", name=f"gr8{b}_{k}")
                            # holds -Gi
                            gi_n = gp.tile([P, KO, N], F8, tag=f"gi8{par}", name=f"gi8{b}_{k}")

                        for m in range(KO):
                            msl = slice(m * P, (m + 1) * P)
                            t_re = ps.tile([P, N], F32, tag="t1", name=f"tre_{b}_{k}_{m}")
                            t_im = ps.tile([P, N], F32, tag="t2", name=f"tim_{b}_{k}_{m}")
                            for dst, pairs in (
                                (t_re, ((xr8_t, r8), (xi8_t, ni8))),
                                (t_im, ((xi8_t, r8), (nxr8_t, ni8))),
                            ):
                                for pi, (x_t, rhs_fn) in enumerate(pairs):
                                    for h in range(2):
                                        nc.tensor.matmul(
                                            dst[:],
                                            lhsT=x_t[:, 2 * h : 2 * h + 2, msl],
                                            rhs=rhs_fn(h),
                                            start=(pi == 0 and h == 0),
                                            stop=(pi == 1 and h == 1),
                                            perf_mode=DR,
                                        )

                            # ScalarE evacuates both banks (folding 1/XSCALE);
                            # VectorE adds the C tiles (ct_i rows are
                            # host-negated for fp8-output steps).
                            s_re = sp.tile([P, N], F16, tag="t1s", name=f"sre_{b}_{k}_{m}")
                            s_im = sp.tile([P, N], F16, tag="t2s", name=f"sim_{b}_{k}_{m}")
                            nc.scalar.activation(
                                s_re[:], t_re[:],
                                mybir.ActivationFunctionType.Copy, scale=1.0 / XSCALE,
                            )
                            nc.scalar.activation(
                                s_im[:], t_im[:],
                                mybir.ActivationFunctionType.Copy, scale=1.0 / XSCALE,
                            )
                            nc.vector.tensor_add(gr_n[:, m, :], s_re[:], ct_r[:, j, m, :])
                            if to_fp16:
                                nc.vector.tensor_add(
                                    gi_n[:, m, :], s_im[:], ct_i[:, j, m, :]
                                )
                                nc.gpsimd.tensor_add(
                                    gs_n[:, m, :], gr_n[:, m, :], gi_n[:, m, :]
                                )
                            else:
                                # -Gi' = (-Ci) - im
                                nc.vector.tensor_sub(
                                    gi_n[:, m, :], ct_i[:, j, m, :], s_im[:]
                                )
                        gcur[b] = (gr_n, gi_n, gs_n) if to_fp16 else (gr_n, gi_n)
                        continue

                    # ---- fp16 Karatsuba step (k = 2..0) ----
                    last = k == 0
                    _gr, _gi, _gs = gcur[b]
                    gr_n = gp.tile([P, KO, N], F16, tag=f"gr{par}", name=f"gr{b}_{k}")
                    gi_n = gp.tile([P, KO, N], F16, tag=f"gi{par}", name=f"gi{b}_{k}")
                    gs_n = (
                        None
                        if last
                        else gp.tile([P, KO, N], F16, tag=f"gs{par}", name=f"gs{b}_{k}")
                    )

                    for m in range(KO):
                        msl = slice(m * P, (m + 1) * P)
                        t1 = ps.tile([P, N], F32, tag="t1", name=f"t1_{b}_{k}_{m}")
                        t2 = ps.tile([P, N], F32, tag="t2", name=f"t2_{b}_{k}_{m}")
                        t3 = ps.tile([P, N], F32, tag="t3", name=f"t3_{b}_{k}_{m}")
                        for dst, x_t, g_t in ((t1, xr_t, _gr), (t2, xi_t, _gi), (t3, xs_t, _gs)):
                            for ko in range(KO):
                                nc.tensor.matmul(
                                    dst[:],
                                    lhsT=x_t[:, ko, msl],
                                    rhs=g_t[:, ko, :],
                                    start=(ko == 0),
                                    stop=(ko == KO - 1),
                                )

                        t1s = sp.tile([P, N], F16, tag="t1s", name=f"t1s_{b}_{k}_{m}")
                        t2s = sp.tile([P, N], F16, tag="t2s", name=f"t2s_{b}_{k}_{m}")
                        t3s = sp.tile([P, N], F16, tag="t3s", name=f"t3s_{b}_{k}_{m}")
                        nc.scalar.copy(t1s[:], t1[:])
                        nc.scalar.copy(t2s[:], t2[:])
                        nc.scalar.copy(t3s[:], t3[:])

                        v1 = vp.tile([P, N], F16, tag="v1", name=f"v1_{b}_{k}_{m}")
                        v2 = vp.tile([P, N], F16, tag="v2", name=f"v2_{b}_{k}_{m}")
                        w2 = vp.tile([P, N], F16, tag="w2", name=f"w2_{b}_{k}_{m}")
                        nc.vector.tensor_sub(v1[:], t1s[:], t2s[:])
                        nc.vector.tensor_add(gr_n[:, m, :], v1[:], ct_r[:, j, m, :])
                        nc.vector.tensor_sub(v2[:], t3s[:], t1s[:])
                        nc.vector.tensor_sub(w2[:], v2[:], t2s[:])
                        nc.vector.tensor_add(gi_n[:, m, :], w2[:], ct_i[:, j, m, :])
                        if last:
                            nc.sync.dma_start(out=or_d[b, m], in_=gr_n[:, m, :])
                            nc.sync.dma_start(out=oi_d[b, m], in_=gi_n[:, m, :])
                        else:
                            nc.vector.tensor_add(
                                gs_n[:, m, :], gr_n[:, m, :], gi_n[:, m, :]
                            )

                    gcur[b] = (gr_n, gi_n, gs_n)

    nc.finalize()
    return nc


def _get_nc() -> bass.Bass:
    if "nc" not in _NC_CACHE:
        _NC_CACHE["nc"] = _build_nc()
    return _NC_CACHE["nc"]


def _tile_layout(m: np.ndarray) -> np.ndarray:
    """[N, N] row-major -> [P, KO, N] (row r at [r % P, r // P, :])."""
    return np.ascontiguousarray(m.reshape(KO, P, N).transpose(1, 0, 2))


def _prep_inputs(x: np.ndarray, coeffs: np.ndarray):
    """Host-side prep: tile/transpose into the DRAM layouts the kernel wants."""
    import ml_dtypes

    f8 = ml_dtypes.float8_e4m3
    x = np.ascontiguousarray(x, dtype=np.float32)
    coeffs = np.ascontiguousarray(coeffs, dtype=np.float32)

    # [B, P, KO, N]
    xrf = x[:, 0].reshape(B, KO, P, N).transpose(0, 2, 1, 3)
    xif = x[:, 1].reshape(B, KO, P, N).transpose(0, 2, 1, 3)
    xr = np.ascontiguousarray(xrf.astype(np.float16))
    xi = np.ascontiguousarray(xif.astype(np.float16))
    xs = np.ascontiguousarray((xrf + xif).astype(np.float16))
    xr8 = np.ascontiguousarray((xrf * XSCALE).astype(f8))
    xi8 = np.ascontiguousarray((xif * XSCALE).astype(f8))
    nxr8 = np.ascontiguousarray((-xrf * XSCALE).astype(f8))

    crT = coeffs[:, 0].transpose(0, 2, 1)  # [DEG, N, N]
    ciT = coeffs[:, 1].transpose(0, 2, 1)
    ctr = np.empty((DEG - 1, P, KO, N), dtype=np.float16)
    cti = np.empty((DEG - 1, P, KO, N), dtype=np.float16)
    for jj in range(DEG - 1):
        k = DEG - 2 - jj
        # Steps whose output stays fp8 carry the state as (Gr, -Gi), so their
        # Ci^T rows are negated: -Gi' = (-Ci^T) - im_products.
        is_ = -1.0 if (k in FP8_STEPS and (k - 1) in FP8_STEPS) else 1.0
        ctr[jj] = _tile_layout(crT[k]).astype(np.float16)
        cti[jj] = (_tile_layout(ciT[k]) * is_).astype(np.float16)
    g08 = np.empty((2, P, KO, N), dtype=f8)
    g08[0] = _tile_layout(crT[DEG - 1]).astype(f8)
    g08[1] = _tile_layout(-ciT[DEG - 1]).astype(f8)

    in_maps = []
    for c in range(NCORES):
        sl = slice(c * BPC, (c + 1) * BPC)
        in_maps.append(
            {
                "xr8": np.ascontiguousarray(xr8[sl]),
                "xi8": np.ascontiguousarray(xi8[sl]),
                "nxr8": np.ascontiguousarray(nxr8[sl]),
                "xr": np.ascontiguousarray(xr[sl]),
                "xi": np.ascontiguousarray(xi[sl]),
                "xs": np.ascontiguousarray(xs[sl]),
                "ctr": ctr,
                "cti": cti,
                "g08": g08,
            }
        )
    return in_maps


def _assemble_output(results) -> np.ndarray:
    out = np.empty((B, 2, N, N), dtype=np.float32)
    for c in range(NCORES):
        o_r = results[c]["o_r"].reshape(BPC, N, N).astype(np.float32)
        o_i = results[c]["o_i"].reshape(BPC, N, N).astype(np.float32)
        for b in range(BPC):
            out[c * BPC + b, 0] = o_r[b].T
            out[c * BPC + b, 1] = o_i[b].T
    return out


def run_sharded(x: np.ndarray, coeffs: np.ndarray, **run_kwargs):
    """Run the SPMD kernel on 8 cores; returns (output, BassKernelResults)."""
    nc = _get_nc()
    in_maps = _prep_inputs(x, coeffs)
    res = run_bass_kernel_spmd(nc, in_maps, list(range(NCORES)), **run_kwargs)
    return _assemble_output(res.results), res


def kernel(x: np.ndarray, coeffs: np.ndarray) -> np.ndarray:
    out, _ = run_sharded(x, coeffs)
    return out
